# revision 13
# baseline (speedup 1.0000x reference)
"""Trainium2 Bass kernel for nn_CustomGPM (multi-scale temporal CNN + RGCN + actor head).

Strategy (hardcoded for the fixed problem shapes):
  B=64 batch, data-parallel over 8 NeuronCores (8 batch elements per core).
  Host-side (index/relayout/weight-fold work only):
    * fold eval-mode BatchNorms into conv / GCN weights and biases
    * express each temporal conv as band matmuls in a (t,ci)-major layout,
      time-split so every contraction fits 128 partitions; conv1 biases ride
      a constant-ones row so the activation is a single pure-LeakyReLU op
    * turn the per-relation gather/scatter-mean into 4 dense, row-normalized
      500x500 adjacency matrices -> RGCN becomes dense matmuls
    * all large operands pre-rounded to bf16; all small weights packed into
      ONE [128, W] DMA to minimize descriptor traffic
  Device-side per core, all-bf16 matmuls with fp32 PSUM accumulation:
    * conv branches: 7 matmuls per batch element, activations split across
      Scalar and Vector engines
    * RGCN aggregation packs TWO batch elements per stationary operand,
      halving the streamed columns; root rides the same PSUM group
    * small actor head + softmax at the end, fp32 output.
"""

import numpy as np
import ml_dtypes

# ---------------- problem constants (hardcoded per spec) ----------------
B = 64
NCORES = 8
BL = B // NCORES  # 8
C0 = 3
N = 500
T = 50
R = 4
P = 500
H = 128
CF = 20
F = 2 * CF + C0   # 43
FP = 67           # xsml rows: s 0..19, m 20..39, zeros 40..63, l 64..66
NCH = 125
SLOPE = 0.01
EPS = 1e-5

S_TS = 24         # s-conv1 out split
M_TS = 15         # m-conv1 out split
KS = 3
KM = 21
KA = 3 * (S_TS + KS - 1)    # 78
KMA = 3 * (M_TS + KM - 1)   # 105
MS = C0 * S_TS              # 72
MM = C0 * M_TS              # 45

BF16 = ml_dtypes.bfloat16

# mega-packed bf16 weights: (name, K rows, M cols); single [128, W] DMA
MEGA = [
    ('ws1h', KA + 1, MS),        # ones-row bias + band
    ('wm1A', KMA + 1, 2 * MM),
    ('wm1B', KMA + 1, 2 * MM),
    ('ws2a', MS, 40), ('ws2b', MS, 40), ('wm2', 2 * MM, 40),
    ('wallt', FP, R * F),
    ('wrootA', FP, 108), ('wrootB', FP, 108),
    ('wzp2a', FP, 2), ('wzp2b', FP, 2), ('wzg2', 107, 2),
    ('at_h', NCH, 4 * BL),
    ('aw2', H, H), ('aw3', H, P + 2),
    ('idt', 128, 128),
    ('w1cb', NCH, 8 * H),
]
MOFF = {}
_o = 0
for _n, _k, _m in MEGA:
    MOFF[_n] = (_o, _k, _m)
    _o += _m
MEGA_W = _o

_CACHE = {}


def _bf(a):
    return np.ascontiguousarray(np.asarray(a, np.float32).astype(BF16))


def _padx(a):
    """[43, X] -> [67, X]: s rows 0..19, m rows 20..39, l rows 64..66."""
    out = np.zeros((FP,) + a.shape[1:], np.float64)
    out[0:2 * CF] = a[0:2 * CF]
    out[64:64 + C0] = a[2 * CF:F]
    return out


# ======================= host-side parameter folding =======================

def _bn_fold(p):
    g, b, m, v = np.asarray(p, np.float64)
    s = g / np.sqrt(v + EPS)
    return s, b - m * s


def _band_t_major(w_eff, t_len, n_out):
    co, ci, kk = w_eff.shape
    band = np.zeros((3 * t_len, co * n_out), np.float64)
    for c in range(co):
        for j in range(n_out):
            for dt in range(kk):
                band[(j + dt) * 3:(j + dt) * 3 + 3, c * n_out + j] = \
                    w_eff[c, :, dt]
    return band


def _host_fold(inp):
    f32 = lambda x: np.asarray(x, np.float32)

    # ---- conv branch weights ----
    ss, ts_ = _bn_fold(inp['sbn1'])
    ws1_eff = np.asarray(inp['sc1_w'], np.float64)[:, :, 0, :] * ss[:, None, None]
    bs1_eff = ss * np.asarray(inp['sc1_b'], np.float64) + ts_
    sm, tm_ = _bn_fold(inp['mbn1'])
    wm1_eff = np.asarray(inp['mc1_w'], np.float64)[:, :, 0, :] * sm[:, None, None]
    bm1_eff = sm * np.asarray(inp['mc1_b'], np.float64) + tm_

    ws1h = np.zeros((KA + 1, MS), np.float64)
    ws1h[0] = np.repeat(bs1_eff, S_TS)
    ws1h[1:] = _band_t_major(ws1_eff, S_TS + KS - 1, S_TS)
    wm1 = _band_t_major(wm1_eff, M_TS + KM - 1, M_TS)        # [105, 45]
    wm1A = np.zeros((KMA + 1, 2 * MM), np.float64)
    wm1A[0] = np.tile(np.repeat(bm1_eff, M_TS), 2)
    wm1A[1:, 0:MM] = wm1
    wm1B = np.zeros((KMA + 1, 2 * MM), np.float64)
    wm1B[1:, MM:2 * MM] = wm1

    def conv2_fold(w, b, bn):
        w = np.asarray(w, np.float64)[:, :, 0, :]
        s, t_ = _bn_fold(bn)
        return w * s[:, None, None], s * np.asarray(b, np.float64) + t_

    w2s, bs2_eff = conv2_fold(inp['sc2_w'], inp['sc2_b'], inp['sbn2'])
    w2m, bm2_eff = conv2_fold(inp['mc2_w'], inp['mc2_b'], inp['mbn2'])
    # merged s2+m2 psum [40, 500]: s2 -> rows 0..19, m2 -> rows 20..39
    ws2a = np.zeros((MS, 40), np.float64)
    ws2b = np.zeros((MS, 40), np.float64)
    for c1 in range(C0):
        for t in range(S_TS):
            ws2a[c1 * S_TS + t, 0:CF] = w2s[:, c1, t]
            ws2b[c1 * S_TS + t, 0:CF] = w2s[:, c1, S_TS + t]
    wm2 = np.zeros((2 * MM, 40), np.float64)
    for c1 in range(C0):
        for t in range(M_TS):
            wm2[c1 * M_TS + t, CF:40] = w2m[:, c1, t]
            wm2[MM + c1 * M_TS + t, CF:40] = w2m[:, c1, M_TS + t]

    # ---- RGCN weights ----
    sg, tg = _bn_fold(inp['gbn'])
    w_all = np.concatenate(
        [np.asarray(inp['gw_rel'], np.float64)[r] * sg[None, :] for r in range(R)],
        axis=1)
    w_root = np.asarray(inp['gw_root'], np.float64) * sg[None, :]
    gb_eff = np.asarray(inp['g_b'], np.float64) * sg + tg

    a_cw = np.asarray(inp['a_cw'], np.float64)
    w_z = a_cw[1:1 + 2 * F]
    wzp = _padx(w_z[:F].reshape(F, 1))
    wrootA = np.zeros((FP, 108), np.float64)
    wrootA[:, 0:F] = _padx(w_root)
    wrootB = np.zeros((FP, 108), np.float64)
    wrootB[:, 64:64 + F] = _padx(w_root)
    wzp2a = np.zeros((FP, 2), np.float64); wzp2a[:, 0:1] = wzp
    wzp2b = np.zeros((FP, 2), np.float64); wzp2b[:, 1:2] = wzp
    wzg2 = np.zeros((107, 2), np.float64)
    wzg2[0:F, 0] = w_z[F:]
    wzg2[64:64 + F, 1] = w_z[F:]

    # ---- adjacency ----
    src = np.asarray(inp['edge_index'][0]).astype(np.int64)
    dst = np.asarray(inp['edge_index'][1]).astype(np.int64)
    etype = np.asarray(inp['edge_type']).astype(np.int64)
    att = []
    for r in range(R):
        sel = etype == r
        cnt = np.zeros((N, N), np.float64)
        np.add.at(cnt, (dst[sel], src[sel]), 1.0)
        deg = cnt.sum(axis=1)
        a_tr = (cnt / np.maximum(deg, 1.0)[:, None]).T
        att.append(_bf(a_tr.reshape(4, NCH, N).transpose(1, 0, 2)
                       .reshape(NCH, 4 * N)))

    # ---- actor head folds ----
    a_cb = float(np.asarray(inp['a_cb'], np.float64)[0])
    a_w1 = np.asarray(inp['a_w1'], np.float64)
    sel_nodes = np.asarray(inp['nodes_to_select']).astype(np.int64)
    w1z = np.zeros((N, H), np.float64)
    np.add.at(w1z, sel_nodes, a_w1[1:])
    w1a = a_cw[0] * a_w1[1:]
    b1_eff = np.asarray(inp['a_b1'], np.float64) + a_cb * a_w1[1:].sum(axis=0)
    w1cat = np.concatenate([w1z, w1a], axis=0)
    w1cb = w1cat.reshape(8, NCH, H).transpose(1, 0, 2).reshape(NCH, 8 * H)

    aw3p = np.zeros((H, P + 2), np.float64)
    aw3p[:, 0:P + 1] = np.asarray(inp['a_w3'], np.float64)

    # ---- mega pack (per-core at_h patched in later) ----
    mega = np.zeros((128, MEGA_W), np.float32)
    vals = {
        'ws1h': ws1h, 'wm1A': wm1A, 'wm1B': wm1B,
        'ws2a': ws2a, 'ws2b': ws2b, 'wm2': wm2,
        'wallt': _padx(w_all), 'wrootA': wrootA, 'wrootB': wrootB,
        'wzp2a': wzp2a, 'wzp2b': wzp2b, 'wzg2': wzg2,
        'at_h': np.zeros((NCH, 4 * BL)), 'aw2': np.asarray(inp['a_w2']),
        'aw3': aw3p, 'idt': np.eye(128), 'w1cb': w1cb,
    }
    for name, k, m in MEGA:
        o = MOFF[name][0]
        mega[0:k, o:o + m] = vals[name]
    mega = _bf(mega)

    # ---- fp32 pack [128, 505]: biases cols 0..3, b3 rows 0..7 cols 4.. ----
    fpk = np.zeros((128, 4 + P + 1), np.float32)
    fpk[0:CF, 0] = bs2_eff
    fpk[CF:40, 0] = bm2_eff
    fpk[0:F, 1] = gb_eff
    fpk[0:H, 2] = b1_eff
    fpk[0:H, 3] = f32(inp['a_b2'])
    fpk[0:BL, 4:] = np.broadcast_to(f32(inp['a_b3']).reshape(1, P + 1),
                                    (BL, P + 1))

    return {
        'mega': mega, 'fpk': fpk,
        'att0': att[0], 'att1': att[1], 'att2': att[2], 'att3': att[3],
    }


# ============================ device kernel ============================

def _build_nc():
    import concourse.bacc as bacc
    import concourse.tile as tile
    import concourse.mybir as mybir

    F32 = mybir.dt.float32
    BF = mybir.dt.bfloat16
    AF = mybir.ActivationFunctionType
    ALU = mybir.AluOpType
    AX = mybir.AxisListType

    nc = bacc.Bacc("TRN2", target_bir_lowering=False, debug=False)

    def din(name, shape, dt=BF):
        return nc.dram_tensor(name, list(shape), dt, kind="ExternalInput").ap()

    HN_ = BL * N // 2
    ON2 = BL * 4 * C0 * T // 2
    obs_ta_d = [din(f'obs_ta{h}', (KMA + 1, HN_)) for h in range(2)]
    obs_tbm_d = [din(f'obs_tbm{h}', (KMA + 1, HN_)) for h in range(2)]
    obs_tbs_d = [din(f'obs_tbs{h}', (KA + 1, HN_)) for h in range(2)]
    obs_n_d = [din(f'obs_n{h}', (NCH, ON2)) for h in range(2)]
    mega_d = din('mega', (128, MEGA_W))
    fpk_d = din('fpk', (128, 4 + P + 1), F32)
    att_d = [din(f'att{r}', (NCH, 4 * N)) for r in range(R)]
    out_d = nc.dram_tensor('out', [BL, P + 1], F32, kind="ExternalOutput").ap()

    mm = nc.tensor.matmul
    HN = BL * N // 2  # 2000, column half for split DMAs

    with tile.TileContext(nc) as tc:
        with tc.tile_pool(name="const", bufs=1) as cp, \
             tc.tile_pool(name="pw", bufs=3) as pw, \
             tc.tile_pool(name="pv", bufs=2) as pv:

            # --- memsets first: no DMA dependency ---
            wt_warm = cp.tile([128, 512], BF, name='wt_warm', tag='wt_warm')
            nc.vector.memset(wt_warm[:], 1.0)
            xsml = [cp.tile([FP, N], BF, name=f'xsml{b}', tag=f'xsml{b}')
                    for b in range(BL)]
            for b in range(BL):
                nc.vector.memset(xsml[b][:], 0.0)

            # --- sync queue: obs slabs as per-half tiles (early start) ---
            ta = [cp.tile([KMA + 1, HN], BF, name=f'ta{h}', tag=f'ta{h}')
                  for h in range(2)]
            tbs = [cp.tile([KA + 1, HN], BF, name=f'tbs{h}', tag=f'tbs{h}')
                   for h in range(2)]
            tbm = [cp.tile([KMA + 1, HN], BF, name=f'tbm{h}', tag=f'tbm{h}')
                   for h in range(2)]
            onat = [cp.tile([NCH, ON2], BF, name=f'onat{h}', tag=f'onat{h}')
                    for h in range(2)]
            for h in range(2):
                nc.sync.dma_start(out=ta[h][:], in_=obs_ta_d[h])
                nc.sync.dma_start(out=tbs[h][:], in_=obs_tbs_d[h])
                nc.sync.dma_start(out=tbm[h][:], in_=obs_tbm_d[h])
            for h in range(2):
                nc.sync.dma_start(out=onat[h][:], in_=obs_n_d[h])

            # --- gpsimd queue: mega weights, half the adjacency ---
            mega = cp.tile([128, MEGA_W], BF, name='mega', tag='mega')
            nc.gpsimd.dma_start(out=mega[:], in_=mega_d)
            att = [cp.tile([NCH, 4 * N], BF, name=f'att{r}', tag=f'att{r}')
                   for r in range(R)]
            nc.gpsimd.dma_start(out=att[0][:], in_=att_d[0])
            nc.gpsimd.dma_start(out=att[1][:], in_=att_d[1])

            # --- scalar queue: fp32 pack + other half of adjacency ---
            fpk = cp.tile([128, 4 + P + 1], F32, name='fpk', tag='fpk')
            nc.scalar.dma_start(out=fpk[:], in_=fpk_d)
            nc.scalar.dma_start(out=att[2][:], in_=att_d[2])
            nc.scalar.dma_start(out=att[3][:], in_=att_d[3])

            def mw(name):
                o, k, m = MOFF[name]
                return mega[0:k, o:o + m]

            # persistent intermediates
            lm_all = cp.tile([NCH, BL * 12], BF, name='lm_all', tag='lm_all')
            hsb = [[cp.tile([NCH, 4 * 107], BF, name=f'h{p}_{c}',
                            tag=f'h{p}_{c}')
                    for c in range(4)] for p in range(BL // 2)]
            for p in range(BL // 2):
                for c in range(4):
                    nc.vector.memset(
                        hsb[p][c][:].rearrange("q (r f) -> q r f", r=4)
                        [:, :, F:64], 0.0)
            ztc = [cp.tile([NCH, BL], BF, name=f'ztc{c}', tag=f'ztc{c}')
                   for c in range(4)]

            # ================= phase 1: warmup + conv =================
            with tc.tile_pool(name="pwm", bufs=2, space="PSUM") as pwm_p, \
                 tc.tile_pool(name="pcb", bufs=4, space="PSUM") as pcb, \
                 tc.tile_pool(name="pcs", bufs=2, space="PSUM") as pcs:

                for w in range(10):
                    pwm = pwm_p.tile([128, 512], F32, name=f'pwm{w}',
                                     tag='pwm')
                    mm(pwm[:], wt_warm[:, 0:128], wt_warm[:], start=True,
                       stop=True)

                s1a_s = [None] * BL
                s1b_s = [None] * BL
                m1_s = [None] * BL

                def lrelu_2op(out_tile, tmp_tile, psum_ap):
                    # DVE lrelu: psum -> bf16 tmp, out = max(tmp*SLOPE, tmp)
                    nc.vector.tensor_copy(tmp_tile[:], psum_ap)
                    nc.vector.scalar_tensor_tensor(
                        out=out_tile[:], in0=tmp_tile[:], scalar=SLOPE,
                        in1=tmp_tile[:], op0=ALU.mult, op1=ALU.max)

                def conv1(b):
                    h, bs = b // 4, slice((b % 4) * N, (b % 4 + 1) * N)
                    ps1a = pcb.tile([MS, N], F32, name=f'ps1a{b}', tag='pcb')
                    mm(ps1a[:], mw('ws1h'), ta[h][0:KA + 1, bs], start=True,
                       stop=True)
                    ps1b = pcb.tile([MS, N], F32, name=f'ps1b{b}', tag='pcb')
                    mm(ps1b[:], mw('ws1h'), tbs[h][:, bs], start=True,
                       stop=True)
                    pm1 = pcb.tile([2 * MM, N], F32, name=f'pm1{b}', tag='pcb')
                    mm(pm1[:], mw('wm1A'), ta[h][:, bs], start=True,
                       stop=False)
                    mm(pm1[:], mw('wm1B'), tbm[h][:, bs], start=False,
                       stop=True)
                    s1a = pw.tile([MS, N], BF, name=f's1a{b}', tag='s1a')
                    s1b = pw.tile([MS, N], BF, name=f's1b{b}', tag='s1b')
                    tmpa = pw.tile([MS, N], BF, name=f'tmpa{b}', tag='tmpa')
                    m1 = pw.tile([2 * MM, N], BF, name=f'm1{b}', tag='m1')
                    lrelu_2op(s1a, tmpa, ps1a[:])
                    if b % 2 == 0:
                        nc.scalar.activation(s1b[:], ps1b[:], AF.Lrelu,
                                             alpha=SLOPE)
                    else:
                        tmpb = pw.tile([MS, N], BF, name=f'tmpb{b}',
                                       tag='tmpb')
                        lrelu_2op(s1b, tmpb, ps1b[:])
                    nc.scalar.activation(m1[:], pm1[:], AF.Lrelu, alpha=SLOPE)
                    s1a_s[b], s1b_s[b], m1_s[b] = s1a, s1b, m1

                def conv2(b):
                    p2 = pcs.tile([40, N], F32, name=f'p2{b}', tag='pcs')
                    mm(p2[:], mw('ws2a'), s1a_s[b][:], start=True, stop=False)
                    mm(p2[:], mw('ws2b'), s1b_s[b][:], start=False, stop=False)
                    mm(p2[:], mw('wm2'), m1_s[b][:], start=False, stop=True)
                    nc.scalar.activation(xsml[b][0:40, :], p2[:], AF.Lrelu,
                                         bias=fpk[0:40, 0:1], alpha=SLOPE)

                conv1(0)
                for b in range(BL):
                    if b + 1 < BL:
                        conv1(b + 1)
                    conv2(b)

            # ================= phase 1.5: lmax =================
            with tc.tile_pool(name="plm", bufs=2, space="PSUM") as plm:
                for bb in range(BL):
                    nc.vector.tensor_reduce(
                        lm_all[:, bb * 12:(bb + 1) * 12].rearrange(
                            "p (c k) -> p c k", c=4),
                        onat[bb // 4][:, (bb % 4) * 600:
                                      (bb % 4 + 1) * 600].rearrange(
                            "p (c k t) -> p c k t", c=4, k=C0),
                        axis=AX.X, op=ALU.max)
                for bb in range(BL):
                    pt = plm.tile([C0, 512], BF, name=f'pt{bb}', tag='pt')
                    for c in range(4):
                        nc.tensor.transpose(
                            pt[:, c * 128:c * 128 + NCH],
                            lm_all[:, bb * 12 + c * 3:bb * 12 + c * 3 + 3],
                            mw('idt')[0:NCH, 0:NCH])
                    nc.scalar.activation(
                        xsml[bb][64:64 + C0, :].rearrange(
                            "p (c n) -> p c n", c=4),
                        pt[:].rearrange("p (c n) -> p c n", c=4)[:, :, 0:NCH],
                        AF.Lrelu, alpha=SLOPE)

            # ================= phase 2: H = x @ W_rel =================
            with tc.tile_pool(name="pph", bufs=4, space="PSUM") as pph:
                for b in range(BL):
                    p, which = b // 2, b % 2
                    off = 0 if which == 0 else 64
                    for c in range(4):
                        ph = pph.tile([NCH, R * F], F32, name=f'ph{b}{c}',
                                      tag='ph')
                        mm(ph[:], xsml[b][:, c * NCH:(c + 1) * NCH],
                           mw('wallt'), start=True, stop=True)
                        dst = hsb[p][c][:].rearrange(
                            "q (r f) -> q r f", r=4)[:, :, off:off + F]
                        src = ph[:].rearrange("q (r f) -> q r f", r=4)
                        if c < 2:
                            nc.vector.tensor_copy(dst, src)
                        else:
                            nc.scalar.activation(dst, src, AF.Copy)

            # ============ phase 3: pairs (root+agg+z) + head ============
            with tc.tile_pool(name="ppg", bufs=2, space="PSUM") as ppg, \
                 tc.tile_pool(name="ppz", bufs=2, space="PSUM") as ppz, \
                 tc.tile_pool(name="pptz", bufs=1, space="PSUM") as pptz:

                ptz = [pptz.tile([NCH, BL], BF, name=f'ptz{c}', tag=f'ptz{c}')
                       for c in range(4)]
                xg_s = [None] * 4
                pg_s = [None] * 4

                def agg_group(p):
                    b0, b1 = 2 * p, 2 * p + 1
                    pg = ppg.tile([107, N], F32, name=f'pg{p}', tag='pg')
                    mm(pg[:], mw('wrootA')[:, 0:107], xsml[b0][:],
                       start=True, stop=False)
                    mm(pg[:], mw('wrootB')[:, 0:107], xsml[b1][:],
                       start=False, stop=False)
                    for r in range(R):
                        for c in range(4):
                            last = (r == R - 1 and c == 3)
                            mm(pg[:],
                               hsb[p][c][:, r * 107:(r + 1) * 107],
                               att[r][:, c * N:(c + 1) * N],
                               start=False, stop=last)
                    xg = pw.tile([107, N], BF, name=f'xg{p}', tag='xg')
                    nc.gpsimd.memset(xg[32:64, :], 0.0)
                    nc.scalar.activation(xg[0:F, :], pg[0:F, :], AF.Lrelu,
                                         bias=fpk[0:F, 1:2], alpha=SLOPE)
                    nc.scalar.activation(xg[64:64 + F, :], pg[64:64 + F, :],
                                         AF.Lrelu, bias=fpk[0:F, 1:2],
                                         alpha=SLOPE)
                    pg_s[p], xg_s[p] = pg, xg

                def z_tail(p):
                    b0, b1 = 2 * p, 2 * p + 1
                    pz = ppz.tile([2, N], F32, name=f'pz{p}', tag='pz')
                    mm(pz[:], mw('wzp2a'), xsml[b0][:], start=True, stop=False)
                    mm(pz[:], mw('wzp2b'), xsml[b1][:], start=False,
                       stop=False)
                    mm(pz[:], mw('wzg2'), xg_s[p][:], start=False, stop=True)
                    zp = pw.tile([2, N], BF, name=f'zp{p}', tag='zp')
                    nc.vector.tensor_copy(zp[:], pz[:])
                    for c in range(4):
                        nc.tensor.transpose(
                            ptz[c][:, 2 * p:2 * p + 2],
                            zp[:, c * NCH:(c + 1) * NCH], mw('idt')[0:2, 0:2])

                agg_group(0)
                for p in range(4):
                    if p + 1 < 4:
                        agg_group(p + 1)
                    z_tail(p)

                for c in range(4):
                    nc.vector.tensor_copy(ztc[c][:], ptz[c][:])

                # ---- actor head ----
                pg1 = ppz.tile([H, BL], F32, name='pg1', tag='pz')
                for c in range(8):
                    rhs = (ztc[c][:] if c < 4 else
                           mw('at_h')[:, (c - 4) * BL:(c - 3) * BL])
                    mm(pg1[:], mw('w1cb')[:, c * H:(c + 1) * H], rhs,
                       start=(c == 0), stop=(c == 7))
                g1 = pv.tile([H, BL], BF, name='g1', tag='g1')
                nc.scalar.activation(g1[:], pg1[:], AF.Relu,
                                     bias=fpk[0:H, 2:3])
                pg2 = ppz.tile([H, BL], F32, name='pg2', tag='pz')
                mm(pg2[:], mw('aw2'), g1[:], start=True, stop=True)
                g2 = pv.tile([H, BL], BF, name='g2', tag='g2')
                nc.scalar.activation(g2[:], pg2[:], AF.Relu,
                                     bias=fpk[0:H, 3:4])
                po_ = ppz.tile([BL, P + 1], F32, name='po_', tag='pz')
                mm(po_[:], g2[:], mw('aw3')[:, 0:P + 1], start=True, stop=True)

                # softmax over free dim (logits = po_ + b3)
                sh = pv.tile([BL, P + 1], F32, name='sh', tag='sh')
                nc.vector.tensor_tensor(out=sh[:], in0=po_[:],
                                        in1=fpk[0:BL, 4:4 + P + 1],
                                        op=ALU.add)
                mx = pv.tile([BL, 1], F32, name='mx', tag='mx')
                nc.vector.tensor_reduce(mx[:], sh[:], axis=AX.X, op=ALU.max)
                sh2 = pv.tile([BL, P + 1], F32, name='sh2', tag='sh2')
                nc.vector.tensor_scalar(sh2[:], sh[:], mx[:, 0:1], None,
                                        op0=ALU.subtract)
                ex = pv.tile([BL, P + 1], F32, name='ex', tag='ex')
                sm = pv.tile([BL, 1], F32, name='sm', tag='sm')
                nc.scalar.activation(ex[:], sh2[:], AF.Exp,
                                     accum_out=sm[:, 0:1])
                rc = pv.tile([BL, 1], F32, name='rc', tag='rc')
                nc.vector.reciprocal(rc[:], sm[:])
                res = pv.tile([BL, P + 1], F32, name='res', tag='res')
                nc.vector.tensor_scalar(res[:], ex[:], rc[:, 0:1], None,
                                        op0=ALU.mult)
                nc.sync.dma_start(out=out_d[:], in_=res[:])

    nc.compile()
    return nc


def _get_nc():
    if 'nc' not in _CACHE:
        _CACHE['nc'] = _build_nc()
    return _CACHE['nc']


# ============================ entry point ============================

def _shard_inputs(inputs):
    folded = _host_fold(inputs)
    obs = np.asarray(inputs['observation'], np.float32)
    action = np.asarray(inputs['action'], np.float32)

    obs_f = _bf(obs.transpose(0, 3, 1, 2))                    # [B, T, C0, N]
    obs_f = obs_f.reshape(B, T * C0, N)
    obs_nm = _bf(obs.transpose(2, 0, 1, 3)
                 .reshape(4, NCH, B, C0, T).transpose(1, 2, 0, 3, 4))
    act_b = _bf(action[:, 1:])
    ones = np.ones((1, BL * N), BF16)
    at_off = MOFF['at_h'][0]

    in_maps = []
    for i in range(NCORES):
        bs = slice(i * BL, (i + 1) * BL)
        of = np.ascontiguousarray(
            obs_f[bs].transpose(1, 0, 2)).reshape(T * C0, BL * N)
        mega = folded['mega'].copy()
        mega[0:NCH, at_off:at_off + 4 * BL] = (
            act_b[bs].reshape(BL, 4, NCH).transpose(2, 1, 0)
            .reshape(NCH, 4 * BL))
        m = {
            'mega': mega, 'fpk': folded['fpk'],
            'att0': folded['att0'], 'att1': folded['att1'],
            'att2': folded['att2'], 'att3': folded['att3'],
        }
        ta_full = np.concatenate([ones, of[0:KMA]], axis=0)
        tbm_full = np.concatenate([ones, of[3 * M_TS:3 * M_TS + KMA]], axis=0)
        tbs_full = np.concatenate([ones, of[3 * S_TS:3 * S_TS + KA]], axis=0)
        on_full = np.ascontiguousarray(
            obs_nm[:, bs]).reshape(NCH, BL * 4 * C0 * T)
        hn = BL * N // 2
        on2 = BL * 4 * C0 * T // 2
        for h in range(2):
            m[f'obs_ta{h}'] = np.ascontiguousarray(
                ta_full[:, h * hn:(h + 1) * hn])
            m[f'obs_tbm{h}'] = np.ascontiguousarray(
                tbm_full[:, h * hn:(h + 1) * hn])
            m[f'obs_tbs{h}'] = np.ascontiguousarray(
                tbs_full[:, h * hn:(h + 1) * hn])
            m[f'obs_n{h}'] = np.ascontiguousarray(
                on_full[:, h * on2:(h + 1) * on2])
        in_maps.append(m)
    return in_maps


def kernel(**inputs) -> np.ndarray:
    from concourse.bass_utils import run_bass_kernel_spmd

    in_maps = _shard_inputs(inputs)
    nc = _get_nc()
    res = run_bass_kernel_spmd(nc, in_maps, list(range(NCORES)))
    return np.concatenate([r['out'] for r in res.results], axis=0)


# revision 16
# speedup vs baseline: 1.2471x; 1.2471x over previous
"""Trainium2 Bass kernel for nn_CustomGPM (multi-scale temporal CNN + RGCN + actor head).

Strategy (hardcoded for the fixed problem shapes):
  B=64 batch, data-parallel over 8 NeuronCores (8 batch elements per core).
  Host-side (index/relayout/weight-fold work only):
    * fold eval-mode BatchNorms into conv / GCN weights and biases
    * express each temporal conv as band matmuls in a (t,ci)-major layout,
      time-split so every contraction fits 128 partitions; conv1 biases ride
      a constant-ones row so the activation is a single pure-LeakyReLU op
    * turn the per-relation gather/scatter-mean into 4 dense, row-normalized
      500x500 adjacency matrices -> RGCN becomes dense matmuls
    * all large operands pre-rounded to bf16; all small weights packed into
      ONE [128, W] DMA to minimize descriptor traffic
  Device-side per core, all-bf16 matmuls with fp32 PSUM accumulation:
    * conv branches: 7 matmuls per batch element, activations split across
      Scalar and Vector engines
    * RGCN aggregation packs TWO batch elements per stationary operand,
      halving the streamed columns; root rides the same PSUM group
    * small actor head + softmax at the end, fp32 output.
"""

import numpy as np
import ml_dtypes

# ---------------- problem constants (hardcoded per spec) ----------------
B = 64
NCORES = 8
BL = B // NCORES  # 8
C0 = 3
N = 500
T = 50
R = 4
P = 500
H = 128
CF = 20
F = 2 * CF + C0   # 43
FP = 67           # xsml rows: s 0..19, m 20..39, zeros 40..63, l 64..66
NCH = 125
SLOPE = 0.01
EPS = 1e-5

SA_TS = 15        # s-conv1 out split: [0,15) from TA, [15,48) from TBM slab
M_TS = 15         # m-conv1 out split: [0,15) from TA, [15,30) from TBM
KS = 3
KM = 21
KSA = 3 * (SA_TS + KS - 1)   # 51  (s1a contraction, no ones row)
KSB = 3 * (48 - SA_TS + KS - 1)  # 105 (s1b contraction == KMA)
KMA = 3 * (M_TS + KM - 1)    # 105
MSA = C0 * SA_TS             # 45 (s1a out cols)
MSB = C0 * (48 - SA_TS)      # 99 (s1b out cols)
MM = C0 * M_TS               # 45

BF16 = ml_dtypes.bfloat16

# mega-packed bf16 weights; two DMAs: conv part (early) + graph/head part
MEGA_C = [
    ('ws1a', KSA + 1, MSA),
    ('ws1b', KSB + 1, MSB),
    ('wm1A', KMA + 1, 2 * MM),
    ('wm1B', KMA + 1, 2 * MM),
    ('ws2a', MSA, 40), ('ws2b', MSB, 40), ('wm2', 2 * MM, 40),
]
MEGA_G = [
    ('wallt', FP, R * F),
    ('wrootA', FP, 108), ('wrootB', FP, 108),
    ('wzp2a', FP, 2), ('wzp2b', FP, 2), ('wzg2', 107, 2),
    ('at_h', NCH, 4 * BL),
    ('aw2', H, H), ('aw3', H, P + 2),
    ('idt', 128, 128),
    ('w1cb', NCH, 8 * H),
]
MOFF = {}
_o = 0
for _n, _k, _m in MEGA_C:
    MOFF[_n] = ('c', _o, _k, _m)
    _o += _m
MEGA_CW = _o
_o = 0
for _n, _k, _m in MEGA_G:
    MOFF[_n] = ('g', _o, _k, _m)
    _o += _m
MEGA_GW = _o

_CACHE = {}


def _bf(a):
    return np.ascontiguousarray(np.asarray(a, np.float32).astype(BF16))


def _padx(a):
    """[43, X] -> [67, X]: s rows 0..19, m rows 20..39, l rows 64..66."""
    out = np.zeros((FP,) + a.shape[1:], np.float64)
    out[0:2 * CF] = a[0:2 * CF]
    out[64:64 + C0] = a[2 * CF:F]
    return out


# ======================= host-side parameter folding =======================

def _bn_fold(p):
    g, b, m, v = np.asarray(p, np.float64)
    s = g / np.sqrt(v + EPS)
    return s, b - m * s


def _band_t_major(w_eff, t_len, n_out):
    co, ci, kk = w_eff.shape
    band = np.zeros((3 * t_len, co * n_out), np.float64)
    for c in range(co):
        for j in range(n_out):
            for dt in range(kk):
                band[(j + dt) * 3:(j + dt) * 3 + 3, c * n_out + j] = \
                    w_eff[c, :, dt]
    return band


def _host_fold(inp):
    f32 = lambda x: np.asarray(x, np.float32)

    # ---- conv branch weights ----
    ss, ts_ = _bn_fold(inp['sbn1'])
    ws1_eff = np.asarray(inp['sc1_w'], np.float64)[:, :, 0, :] * ss[:, None, None]
    bs1_eff = ss * np.asarray(inp['sc1_b'], np.float64) + ts_
    sm, tm_ = _bn_fold(inp['mbn1'])
    wm1_eff = np.asarray(inp['mc1_w'], np.float64)[:, :, 0, :] * sm[:, None, None]
    bm1_eff = sm * np.asarray(inp['mc1_b'], np.float64) + tm_

    ws1a = np.zeros((KSA + 1, MSA), np.float64)
    ws1a[0] = np.repeat(bs1_eff, SA_TS)
    ws1a[1:] = _band_t_major(ws1_eff, SA_TS + KS - 1, SA_TS)
    ws1b = np.zeros((KSB + 1, MSB), np.float64)
    ws1b[0] = np.repeat(bs1_eff, 48 - SA_TS)
    ws1b[1:] = _band_t_major(ws1_eff, 48 - SA_TS + KS - 1, 48 - SA_TS)
    wm1 = _band_t_major(wm1_eff, M_TS + KM - 1, M_TS)        # [105, 45]
    wm1A = np.zeros((KMA + 1, 2 * MM), np.float64)
    wm1A[0] = np.tile(np.repeat(bm1_eff, M_TS), 2)
    wm1A[1:, 0:MM] = wm1
    wm1B = np.zeros((KMA + 1, 2 * MM), np.float64)
    wm1B[1:, MM:2 * MM] = wm1

    def conv2_fold(w, b, bn):
        w = np.asarray(w, np.float64)[:, :, 0, :]
        s, t_ = _bn_fold(bn)
        return w * s[:, None, None], s * np.asarray(b, np.float64) + t_

    w2s, bs2_eff = conv2_fold(inp['sc2_w'], inp['sc2_b'], inp['sbn2'])
    w2m, bm2_eff = conv2_fold(inp['mc2_w'], inp['mc2_b'], inp['mbn2'])
    # merged s2+m2 psum [40, 500]: s2 -> rows 0..19, m2 -> rows 20..39
    ws2a = np.zeros((MSA, 40), np.float64)
    ws2b = np.zeros((MSB, 40), np.float64)
    for c1 in range(C0):
        for t in range(SA_TS):
            ws2a[c1 * SA_TS + t, 0:CF] = w2s[:, c1, t]
        for t in range(48 - SA_TS):
            ws2b[c1 * (48 - SA_TS) + t, 0:CF] = w2s[:, c1, SA_TS + t]
    wm2 = np.zeros((2 * MM, 40), np.float64)
    for c1 in range(C0):
        for t in range(M_TS):
            wm2[c1 * M_TS + t, CF:40] = w2m[:, c1, t]
            wm2[MM + c1 * M_TS + t, CF:40] = w2m[:, c1, M_TS + t]

    # ---- RGCN weights ----
    sg, tg = _bn_fold(inp['gbn'])
    w_all = np.concatenate(
        [np.asarray(inp['gw_rel'], np.float64)[r] * sg[None, :] for r in range(R)],
        axis=1)
    w_root = np.asarray(inp['gw_root'], np.float64) * sg[None, :]
    gb_eff = np.asarray(inp['g_b'], np.float64) * sg + tg

    a_cw = np.asarray(inp['a_cw'], np.float64)
    w_z = a_cw[1:1 + 2 * F]
    wzp = _padx(w_z[:F].reshape(F, 1))
    wrootA = np.zeros((FP, 108), np.float64)
    wrootA[:, 0:F] = _padx(w_root)
    wrootB = np.zeros((FP, 108), np.float64)
    wrootB[:, 64:64 + F] = _padx(w_root)
    wzp2a = np.zeros((FP, 2), np.float64); wzp2a[:, 0:1] = wzp
    wzp2b = np.zeros((FP, 2), np.float64); wzp2b[:, 1:2] = wzp
    wzg2 = np.zeros((107, 2), np.float64)
    wzg2[0:F, 0] = w_z[F:]
    wzg2[64:64 + F, 1] = w_z[F:]

    # ---- adjacency ----
    src = np.asarray(inp['edge_index'][0]).astype(np.int64)
    dst = np.asarray(inp['edge_index'][1]).astype(np.int64)
    etype = np.asarray(inp['edge_type']).astype(np.int64)
    att = []
    for r in range(R):
        sel = etype == r
        cnt = np.zeros((N, N), np.float64)
        np.add.at(cnt, (dst[sel], src[sel]), 1.0)
        deg = cnt.sum(axis=1)
        a_tr = (cnt / np.maximum(deg, 1.0)[:, None]).T
        att.append(_bf(a_tr.reshape(4, NCH, N).transpose(1, 0, 2)
                       .reshape(NCH, 4 * N)))

    # ---- actor head folds ----
    a_cb = float(np.asarray(inp['a_cb'], np.float64)[0])
    a_w1 = np.asarray(inp['a_w1'], np.float64)
    sel_nodes = np.asarray(inp['nodes_to_select']).astype(np.int64)
    w1z = np.zeros((N, H), np.float64)
    np.add.at(w1z, sel_nodes, a_w1[1:])
    w1a = a_cw[0] * a_w1[1:]
    b1_eff = np.asarray(inp['a_b1'], np.float64) + a_cb * a_w1[1:].sum(axis=0)
    w1cat = np.concatenate([w1z, w1a], axis=0)
    w1cb = w1cat.reshape(8, NCH, H).transpose(1, 0, 2).reshape(NCH, 8 * H)

    aw3p = np.zeros((H, P + 2), np.float64)
    aw3p[:, 0:P + 1] = np.asarray(inp['a_w3'], np.float64)

    # ---- mega packs (per-core at_h patched in later) ----
    vals = {
        'ws1a': ws1a, 'ws1b': ws1b, 'wm1A': wm1A, 'wm1B': wm1B,
        'ws2a': ws2a, 'ws2b': ws2b, 'wm2': wm2,
        'wallt': _padx(w_all), 'wrootA': wrootA, 'wrootB': wrootB,
        'wzp2a': wzp2a, 'wzp2b': wzp2b, 'wzg2': wzg2,
        'at_h': np.zeros((NCH, 4 * BL)), 'aw2': np.asarray(inp['a_w2']),
        'aw3': aw3p, 'idt': np.eye(128), 'w1cb': w1cb,
    }
    mega_c = np.zeros((128, MEGA_CW), np.float32)
    for name, k, m in MEGA_C:
        o = MOFF[name][1]
        mega_c[0:k, o:o + m] = vals[name]
    mega_g = np.zeros((128, MEGA_GW), np.float32)
    for name, k, m in MEGA_G:
        o = MOFF[name][1]
        mega_g[0:k, o:o + m] = vals[name]
    mega_c = _bf(mega_c)
    mega_g = _bf(mega_g)

    # ---- fp32 pack [128, 505]: biases cols 0..3, b3 rows 0..7 cols 4.. ----
    fpk = np.zeros((128, 4 + P + 1), np.float32)
    fpk[0:CF, 0] = bs2_eff
    fpk[CF:40, 0] = bm2_eff
    fpk[0:F, 1] = gb_eff
    fpk[0:H, 2] = b1_eff
    fpk[0:H, 3] = f32(inp['a_b2'])
    fpk[0:BL, 4:] = np.broadcast_to(f32(inp['a_b3']).reshape(1, P + 1),
                                    (BL, P + 1))

    return {
        'mega_c': mega_c, 'mega_g': mega_g, 'fpk': fpk,
        'att0': att[0], 'att1': att[1], 'att2': att[2], 'att3': att[3],
    }


# ============================ device kernel ============================

def _build_nc():
    import concourse.bacc as bacc
    import concourse.tile as tile
    import concourse.mybir as mybir

    F32 = mybir.dt.float32
    BF = mybir.dt.bfloat16
    AF = mybir.ActivationFunctionType
    ALU = mybir.AluOpType
    AX = mybir.AxisListType

    nc = bacc.Bacc("TRN2", target_bir_lowering=False, debug=False)

    def din(name, shape, dt=BF):
        return nc.dram_tensor(name, list(shape), dt, kind="ExternalInput").ap()

    HN_ = BL * N // 2
    ON2 = BL * 4 * C0 * T // 2
    obs_ta_d = [din(f'obs_ta{h}', (KMA + 1, HN_)) for h in range(2)]
    obs_tbm_d = [din(f'obs_tbm{h}', (KMA + 1, HN_)) for h in range(2)]
    obs_n_d = [din(f'obs_n{h}', (NCH, ON2)) for h in range(2)]
    mega_c_d = din('mega_c', (128, MEGA_CW))
    mega_g_d = din('mega_g', (128, MEGA_GW))
    fpk_d = din('fpk', (128, 4 + P + 1), F32)
    att_d = [din(f'att{r}', (NCH, 4 * N)) for r in range(R)]
    out_d = nc.dram_tensor('out', [BL, P + 1], F32, kind="ExternalOutput").ap()

    mm = nc.tensor.matmul
    HN = BL * N // 2  # 2000, column half for split DMAs

    with tile.TileContext(nc) as tc:
        with tc.tile_pool(name="const", bufs=1) as cp, \
             tc.tile_pool(name="pw", bufs=3) as pw, \
             tc.tile_pool(name="pv", bufs=2) as pv:

            # --- memsets first: no DMA dependency ---
            wt_warm = cp.tile([128, 512], BF, name='wt_warm', tag='wt_warm')
            nc.vector.memset(wt_warm[:], 1.0)
            xsml = [cp.tile([FP, N], BF, name=f'xsml{b}', tag=f'xsml{b}')
                    for b in range(BL)]
            for b in range(BL):
                nc.vector.memset(xsml[b][:], 0.0)

            # tiles for the three DMA queues, need-ordered:
            #   gpsimd: mega_c, ta0, ta1, att1
            #   sync:   tbm0, on0, tbm1, on1
            #   scalar: fpk, mega_g, att0, att2, att3
            ta = [cp.tile([KMA + 1, HN], BF, name=f'ta{h}', tag=f'ta{h}')
                  for h in range(2)]
            tbm = [cp.tile([KMA + 1, HN], BF, name=f'tbm{h}', tag=f'tbm{h}')
                   for h in range(2)]
            onat = [cp.tile([NCH, ON2], BF, name=f'onat{h}', tag=f'onat{h}')
                    for h in range(2)]
            mega_c = cp.tile([128, MEGA_CW], BF, name='mega_c', tag='mega_c')
            mega_g = cp.tile([128, MEGA_GW], BF, name='mega_g', tag='mega_g')
            fpk = cp.tile([128, 4 + P + 1], F32, name='fpk', tag='fpk')
            att = [cp.tile([NCH, 4 * N], BF, name=f'att{r}', tag=f'att{r}')
                   for r in range(R)]

            nc.gpsimd.dma_start(out=mega_c[:], in_=mega_c_d)
            nc.gpsimd.dma_start(out=ta[0][:], in_=obs_ta_d[0])
            nc.gpsimd.dma_start(out=ta[1][:], in_=obs_ta_d[1])
            nc.gpsimd.dma_start(out=att[1][:], in_=att_d[1])

            nc.sync.dma_start(out=tbm[0][:], in_=obs_tbm_d[0])
            nc.sync.dma_start(out=onat[0][:], in_=obs_n_d[0])
            nc.sync.dma_start(out=tbm[1][:], in_=obs_tbm_d[1])
            nc.sync.dma_start(out=onat[1][:], in_=obs_n_d[1])

            nc.scalar.dma_start(out=fpk[:], in_=fpk_d)
            nc.scalar.dma_start(out=mega_g[:], in_=mega_g_d)
            nc.scalar.dma_start(out=att[0][:], in_=att_d[0])
            nc.scalar.dma_start(out=att[2][:], in_=att_d[2])
            nc.scalar.dma_start(out=att[3][:], in_=att_d[3])

            def mw(name):
                which, o, k, m = MOFF[name]
                t = mega_c if which == 'c' else mega_g
                return t[0:k, o:o + m]

            # persistent intermediates
            lm_all = cp.tile([NCH, BL * 12], BF, name='lm_all', tag='lm_all')
            hsb = [[cp.tile([NCH, 4 * 107], BF, name=f'h{p}_{c}',
                            tag=f'h{p}_{c}')
                    for c in range(4)] for p in range(BL // 2)]
            for p in range(BL // 2):
                for c in range(4):
                    nc.vector.memset(
                        hsb[p][c][:].rearrange("q (r f) -> q r f", r=4)
                        [:, :, F:64], 0.0)
            ztc = [cp.tile([NCH, BL], BF, name=f'ztc{c}', tag=f'ztc{c}')
                   for c in range(4)]

            # ================= phase 1: warmup + conv =================
            with tc.tile_pool(name="pwm", bufs=2, space="PSUM") as pwm_p, \
                 tc.tile_pool(name="pcb", bufs=4, space="PSUM") as pcb, \
                 tc.tile_pool(name="pcs", bufs=2, space="PSUM") as pcs:

                for w in range(10):
                    pwm = pwm_p.tile([128, 512], F32, name=f'pwm{w}',
                                     tag='pwm')
                    mm(pwm[:], wt_warm[:, 0:128], wt_warm[:], start=True,
                       stop=True)

                s1a_s = [None] * BL
                s1b_s = [None] * BL
                m1_s = [None] * BL

                def lrelu_2op(out_tile, tmp_tile, psum_ap):
                    # DVE lrelu: psum -> bf16 tmp, out = max(tmp*SLOPE, tmp)
                    nc.vector.tensor_copy(tmp_tile[:], psum_ap)
                    nc.vector.scalar_tensor_tensor(
                        out=out_tile[:], in0=tmp_tile[:], scalar=SLOPE,
                        in1=tmp_tile[:], op0=ALU.mult, op1=ALU.max)

                def conv1(b):
                    h, bs = b // 4, slice((b % 4) * N, (b % 4 + 1) * N)
                    ps1a = pcb.tile([MSA, N], F32, name=f'ps1a{b}', tag='pcb')
                    mm(ps1a[:], mw('ws1a'), ta[h][0:KSA + 1, bs], start=True,
                       stop=True)
                    ps1b = pcb.tile([MSB, N], F32, name=f'ps1b{b}', tag='pcb')
                    mm(ps1b[:], mw('ws1b'), tbm[h][:, bs], start=True,
                       stop=True)
                    pm1 = pcb.tile([2 * MM, N], F32, name=f'pm1{b}', tag='pcb')
                    mm(pm1[:], mw('wm1A'), ta[h][:, bs], start=True,
                       stop=False)
                    mm(pm1[:], mw('wm1B'), tbm[h][:, bs], start=False,
                       stop=True)
                    s1a = pw.tile([MSA, N], BF, name=f's1a{b}', tag='s1a')
                    s1b = pw.tile([MSB, N], BF, name=f's1b{b}', tag='s1b')
                    tmpa = pw.tile([MSA, N], BF, name=f'tmpa{b}', tag='tmpa')
                    m1 = pw.tile([2 * MM, N], BF, name=f'm1{b}', tag='m1')
                    lrelu_2op(s1a, tmpa, ps1a[:])
                    if b % 2 == 0:
                        nc.scalar.activation(s1b[:], ps1b[:], AF.Lrelu,
                                             alpha=SLOPE)
                    else:
                        tmpb = pw.tile([MSB, N], BF, name=f'tmpb{b}',
                                       tag='tmpb')
                        lrelu_2op(s1b, tmpb, ps1b[:])
                    nc.scalar.activation(m1[:], pm1[:], AF.Lrelu, alpha=SLOPE)
                    s1a_s[b], s1b_s[b], m1_s[b] = s1a, s1b, m1

                def conv2(b):
                    p2 = pcs.tile([40, N], F32, name=f'p2{b}', tag='pcs')
                    mm(p2[:], mw('ws2a'), s1a_s[b][:], start=True, stop=False)
                    mm(p2[:], mw('ws2b'), s1b_s[b][:], start=False, stop=False)
                    mm(p2[:], mw('wm2'), m1_s[b][:], start=False, stop=True)
                    nc.scalar.activation(xsml[b][0:40, :], p2[:], AF.Lrelu,
                                         bias=fpk[0:40, 0:1], alpha=SLOPE)

                conv1(0)
                for b in range(BL):
                    if b + 1 < BL:
                        conv1(b + 1)
                    conv2(b)

            # ========= phase 2: lmax + H, interleaved by batch half =========
            with tc.tile_pool(name="plm", bufs=2, space="PSUM") as plm, \
                 tc.tile_pool(name="pph", bufs=4, space="PSUM") as pph:

                def lmax(bb):
                    nc.vector.tensor_reduce(
                        lm_all[:, bb * 12:(bb + 1) * 12].rearrange(
                            "p (c k) -> p c k", c=4),
                        onat[bb // 4][:, (bb % 4) * 600:
                                      (bb % 4 + 1) * 600].rearrange(
                            "p (c k t) -> p c k t", c=4, k=C0),
                        axis=AX.X, op=ALU.max)
                    pt = plm.tile([C0, 512], BF, name=f'pt{bb}', tag='pt')
                    for c in range(4):
                        nc.tensor.transpose(
                            pt[:, c * 128:c * 128 + NCH],
                            lm_all[:, bb * 12 + c * 3:bb * 12 + c * 3 + 3],
                            mw('idt')[0:NCH, 0:NCH])
                    nc.scalar.activation(
                        xsml[bb][64:64 + C0, :].rearrange(
                            "p (c n) -> p c n", c=4),
                        pt[:].rearrange("p (c n) -> p c n", c=4)[:, :, 0:NCH],
                        AF.Lrelu, alpha=SLOPE)

                def hphase(b):
                    p, which = b // 2, b % 2
                    off = 0 if which == 0 else 64
                    for c in range(4):
                        ph = pph.tile([NCH, R * F], F32, name=f'ph{b}{c}',
                                      tag='ph')
                        mm(ph[:], xsml[b][:, c * NCH:(c + 1) * NCH],
                           mw('wallt'), start=True, stop=True)
                        dst = hsb[p][c][:].rearrange(
                            "q (r f) -> q r f", r=4)[:, :, off:off + F]
                        src_ = ph[:].rearrange("q (r f) -> q r f", r=4)
                        if c < 2:
                            nc.vector.tensor_copy(dst, src_)
                        else:
                            nc.scalar.activation(dst, src_, AF.Copy)

                for bb in range(4):
                    lmax(bb)
                for b in range(4):
                    hphase(b)
                for bb in range(4, BL):
                    lmax(bb)
                for b in range(4, BL):
                    hphase(b)

            # ============ phase 3: pairs (root+agg+z) + head ============
            with tc.tile_pool(name="ppg", bufs=2, space="PSUM") as ppg, \
                 tc.tile_pool(name="ppz", bufs=2, space="PSUM") as ppz, \
                 tc.tile_pool(name="pptz", bufs=1, space="PSUM") as pptz:

                ptz = [pptz.tile([NCH, BL], BF, name=f'ptz{c}', tag=f'ptz{c}')
                       for c in range(4)]
                xg_s = [None] * 4
                pg_s = [None] * 4

                def agg_group(p):
                    b0, b1 = 2 * p, 2 * p + 1
                    pg = ppg.tile([107, N], F32, name=f'pg{p}', tag='pg')
                    mm(pg[:], mw('wrootA')[:, 0:107], xsml[b0][:],
                       start=True, stop=False)
                    mm(pg[:], mw('wrootB')[:, 0:107], xsml[b1][:],
                       start=False, stop=False)
                    for r in range(R):
                        for c in range(4):
                            last = (r == R - 1 and c == 3)
                            mm(pg[:],
                               hsb[p][c][:, r * 107:(r + 1) * 107],
                               att[r][:, c * N:(c + 1) * N],
                               start=False, stop=last)
                    xg = pw.tile([107, N], BF, name=f'xg{p}', tag='xg')
                    nc.gpsimd.memset(xg[32:64, :], 0.0)
                    nc.scalar.activation(xg[0:F, :], pg[0:F, :], AF.Lrelu,
                                         bias=fpk[0:F, 1:2], alpha=SLOPE)
                    nc.scalar.activation(xg[64:64 + F, :], pg[64:64 + F, :],
                                         AF.Lrelu, bias=fpk[0:F, 1:2],
                                         alpha=SLOPE)
                    pg_s[p], xg_s[p] = pg, xg

                def z_tail(p):
                    b0, b1 = 2 * p, 2 * p + 1
                    pz = ppz.tile([2, N], F32, name=f'pz{p}', tag='pz')
                    mm(pz[:], mw('wzp2a'), xsml[b0][:], start=True, stop=False)
                    mm(pz[:], mw('wzp2b'), xsml[b1][:], start=False,
                       stop=False)
                    mm(pz[:], mw('wzg2'), xg_s[p][:], start=False, stop=True)
                    zp = pw.tile([2, N], BF, name=f'zp{p}', tag='zp')
                    nc.vector.tensor_copy(zp[:], pz[:])
                    for c in range(4):
                        nc.tensor.transpose(
                            ptz[c][:, 2 * p:2 * p + 2],
                            zp[:, c * NCH:(c + 1) * NCH], mw('idt')[0:2, 0:2])

                agg_group(0)
                for p in range(4):
                    if p + 1 < 4:
                        agg_group(p + 1)
                    z_tail(p)

                for c in range(4):
                    nc.vector.tensor_copy(ztc[c][:], ptz[c][:])

                # ---- actor head ----
                pg1 = ppz.tile([H, BL], F32, name='pg1', tag='pz')
                for c in range(8):
                    rhs = (ztc[c][:] if c < 4 else
                           mw('at_h')[:, (c - 4) * BL:(c - 3) * BL])
                    mm(pg1[:], mw('w1cb')[:, c * H:(c + 1) * H], rhs,
                       start=(c == 0), stop=(c == 7))
                g1 = pv.tile([H, BL], BF, name='g1', tag='g1')
                nc.scalar.activation(g1[:], pg1[:], AF.Relu,
                                     bias=fpk[0:H, 2:3])
                pg2 = ppz.tile([H, BL], F32, name='pg2', tag='pz')
                mm(pg2[:], mw('aw2'), g1[:], start=True, stop=True)
                g2 = pv.tile([H, BL], BF, name='g2', tag='g2')
                nc.scalar.activation(g2[:], pg2[:], AF.Relu,
                                     bias=fpk[0:H, 3:4])
                po_ = ppz.tile([BL, P + 1], F32, name='po_', tag='pz')
                mm(po_[:], g2[:], mw('aw3')[:, 0:P + 1], start=True, stop=True)

                # softmax over free dim (logits = po_ + b3)
                sh = pv.tile([BL, P + 1], F32, name='sh', tag='sh')
                nc.vector.tensor_tensor(out=sh[:], in0=po_[:],
                                        in1=fpk[0:BL, 4:4 + P + 1],
                                        op=ALU.add)
                mx = pv.tile([BL, 1], F32, name='mx', tag='mx')
                nc.vector.tensor_reduce(mx[:], sh[:], axis=AX.X, op=ALU.max)
                sh2 = pv.tile([BL, P + 1], F32, name='sh2', tag='sh2')
                nc.vector.tensor_scalar(sh2[:], sh[:], mx[:, 0:1], None,
                                        op0=ALU.subtract)
                ex = pv.tile([BL, P + 1], F32, name='ex', tag='ex')
                sm = pv.tile([BL, 1], F32, name='sm', tag='sm')
                nc.scalar.activation(ex[:], sh2[:], AF.Exp,
                                     accum_out=sm[:, 0:1])
                rc = pv.tile([BL, 1], F32, name='rc', tag='rc')
                nc.vector.reciprocal(rc[:], sm[:])
                res = pv.tile([BL, P + 1], F32, name='res', tag='res')
                nc.vector.tensor_scalar(res[:], ex[:], rc[:, 0:1], None,
                                        op0=ALU.mult)
                nc.sync.dma_start(out=out_d[:], in_=res[:])

    nc.compile()
    return nc


def _get_nc():
    if 'nc' not in _CACHE:
        _CACHE['nc'] = _build_nc()
    return _CACHE['nc']


# ============================ entry point ============================

def _shard_inputs(inputs):
    folded = _host_fold(inputs)
    obs = np.asarray(inputs['observation'], np.float32)
    action = np.asarray(inputs['action'], np.float32)

    obs_f = _bf(obs.transpose(0, 3, 1, 2))                    # [B, T, C0, N]
    obs_f = obs_f.reshape(B, T * C0, N)
    obs_nm = _bf(obs.transpose(2, 0, 1, 3)
                 .reshape(4, NCH, B, C0, T).transpose(1, 2, 0, 3, 4))
    act_b = _bf(action[:, 1:])
    ones = np.ones((1, BL * N), BF16)
    at_off = MOFF['at_h'][1]

    in_maps = []
    for i in range(NCORES):
        bs = slice(i * BL, (i + 1) * BL)
        of = np.ascontiguousarray(
            obs_f[bs].transpose(1, 0, 2)).reshape(T * C0, BL * N)
        mega_g = folded['mega_g'].copy()
        mega_g[0:NCH, at_off:at_off + 4 * BL] = (
            act_b[bs].reshape(BL, 4, NCH).transpose(2, 1, 0)
            .reshape(NCH, 4 * BL))
        m = {
            'mega_c': folded['mega_c'], 'mega_g': mega_g,
            'fpk': folded['fpk'],
            'att0': folded['att0'], 'att1': folded['att1'],
            'att2': folded['att2'], 'att3': folded['att3'],
        }
        ta_full = np.concatenate([ones, of[0:KMA]], axis=0)
        tbm_full = np.concatenate([ones, of[3 * M_TS:3 * M_TS + KMA]], axis=0)
        on_full = np.ascontiguousarray(
            obs_nm[:, bs]).reshape(NCH, BL * 4 * C0 * T)
        hn = BL * N // 2
        on2 = BL * 4 * C0 * T // 2
        for h in range(2):
            m[f'obs_ta{h}'] = np.ascontiguousarray(
                ta_full[:, h * hn:(h + 1) * hn])
            m[f'obs_tbm{h}'] = np.ascontiguousarray(
                tbm_full[:, h * hn:(h + 1) * hn])
            m[f'obs_n{h}'] = np.ascontiguousarray(
                on_full[:, h * on2:(h + 1) * on2])
        in_maps.append(m)
    return in_maps


def kernel(**inputs) -> np.ndarray:
    from concourse.bass_utils import run_bass_kernel_spmd

    in_maps = _shard_inputs(inputs)
    nc = _get_nc()
    res = run_bass_kernel_spmd(nc, in_maps, list(range(NCORES)))
    return np.concatenate([r['out'] for r in res.results], axis=0)
